# revision 31
# baseline (speedup 1.0000x reference)
"""Trainium2 Bass kernel for nn_NeuralNetworkDPD (dense_mlp)  — v2.

Feature-major, 2-token-halves packed on 128 partitions (A rows {0,1},
B rows {2,3}; partitions [0:64)=A-token features, [64:128)=B).

v2 changes vs v1:
  - Mean-centering projection P = I - 11^T/64 folded into every weight
    matrix host-side, so each layer's activations arrive pre-centered:
    the LN mean matmul and the subtract disappear.  The lost means (the
    final  Z @ w_out  needs the uncentered Z) are reconstructed with 4
    tiny matmuls (feats*mean(w_in), u1*mean(W1), u3*mean(W3),
    u5*mean(W5)) accumulated straight into the output PSUM bank with
    colsum(w_out) folded into their weights.
  - bf16 for all SBUF-resident tensors (weights + activations): 2x DVE
    modes, FWL weight loads, faster PE streaming.  PSUM stays fp32.
  - LN variance path: Square(zp + bias) fused on ACT directly from
    PSUM -> va matmul -> Abs_reciprocal_sqrt(va + eps) on ACT (one op,
    one activation table) -> t = (zp + bias) * rs via
    scalar_tensor_tensor on DVE -> Prelu(gamma*t + beta) on ACT.
  - Residual adds fused into the PSUM->SBUF materialize via
    scalar_tensor_tensor: Z = (zp + bias) + Z_prev.
  - Planar DRAM output [4, 2, N] (contiguous stores; real/imag
    interleave done on host) - kills the v1 4-byte-descriptor storm.
  - Inputs converted fp32->bf16 on device (wide reshape + DVE copy),
    feats loaded one dma_start per half per super-chunk.
"""

import sys
from contextlib import ExitStack

sys.path.insert(0, "/opt/trn_rl_repo")

import numpy as np

import concourse.bacc as bacc
import concourse.bass as bass
import concourse.tile as tile
from concourse import mybir

F = 64          # feature width
NL = 6          # chained dense layers
EPS = 1e-3
CH = 512        # tokens per matmul (PSUM bank)
SUP = 8         # chunks per super-chunk (scheduling window)
BF = mybir.dt.bfloat16
FP = mybir.dt.float32
AF = mybir.ActivationFunctionType
OP = mybir.AluOpType

USE_ARS = True  # Abs_reciprocal_sqrt on ACT; False -> Sqrt + DVE recip
USE_GP = False  # GPSIMD offload: measured 8-15us/op + DVE port contention


def build_kernel(tc, outs, ins, tokens_per_row):
    nc = tc.nc
    TPR = tokens_per_row
    cpr = TPR // CH
    spr = cpr // SUP
    assert cpr % SUP == 0
    NG = SUP // 2               # groups (of 2 chunks) per super
    N3 = TPR + 3

    xr, xi = ins["xr"], ins["xi"]       # [4, TPR] fp32
    out = outs["out"]                   # [4, 2, TPR] fp32 planar

    xpad = nc.dram_tensor("xpad", [4, 2, N3], BF, kind="Internal").ap()

    ctx = ExitStack()
    singles = ctx.enter_context(tc.tile_pool(name="singles", bufs=1))
    cvt32 = ctx.enter_context(tc.tile_pool(name="cvt32", bufs=2))
    cvt16 = ctx.enter_context(tc.tile_pool(name="cvt16", bufs=2))
    fpool = ctx.enter_context(tc.tile_pool(name="fpool", bufs=2))
    anchors = ctx.enter_context(tc.tile_pool(name="anchors", bufs=10))
    upool = ctx.enter_context(tc.tile_pool(name="upool", bufs=17))
    qpool = ctx.enter_context(tc.tile_pool(name="qpool", bufs=8))   # vsq/zc
    rpool = ctx.enter_context(tc.tile_pool(name="rpool", bufs=6))   # rs
    tpool = ctx.enter_context(tc.tile_pool(name="tpool", bufs=4))   # t
    opool = ctx.enter_context(tc.tile_pool(name="opool", bufs=4))
    zp_pool = ctx.enter_context(tc.tile_pool(name="zp", bufs=4, space="PSUM"))
    sp_pool = ctx.enter_context(tc.tile_pool(name="sp", bufs=2, space="PSUM"))

    # ---- weights/constants -> SBUF ----
    wd = singles.tile([128, NL * 128], BF)        # folded dense, block-diag
    win = singles.tile([16, 128], BF)             # folded w_in, block-diag
    wout = singles.tile([128, 4], BF)             # w_out block-diag
    mw = singles.tile([128, 12], BF)              # mean-track lhsT l=1,3,5
    mwin = singles.tile([16, 4], BF)              # mean-track lhsT for feats
    onesg = singles.tile([128, NL * 128], BF)     # LN stats lhsT, 1/(64*g^2)
    percol = singles.tile([128, 26], FP)          # per-partition columns
    nc.sync.dma_start(out=wd, in_=ins["wd"])
    nc.sync.dma_start(out=win, in_=ins["win"])
    nc.sync.dma_start(out=wout, in_=ins["wout"])
    nc.sync.dma_start(out=mw, in_=ins["mw"])
    nc.sync.dma_start(out=mwin, in_=ins["mwin"])
    nc.sync.dma_start(out=onesg, in_=ins["onesg"])
    nc.sync.dma_start(out=percol, in_=ins["percol"])

    b_in_col = percol[:, 0:1]
    bcol = [percol[:, 1 + l: 2 + l] for l in range(NL)]       # centered b_l
    gscol = [percol[:, 7 + l: 8 + l] for l in range(NL)]      # eps/gamma^2
    ecol = [percol[:, 13 + l: 14 + l] for l in range(NL)]     # beta
    acol = [percol[:, 19 + l: 20 + l] for l in range(NL)]     # alpha

    # ---- convert x to bf16 into padded DRAM ----
    # [4, TPR] fp32 viewed as [128, TPR//32]: partition p = 32*row + blk
    W = TPR // 32
    zrow = singles.tile([8, 4], BF)
    nc.vector.memset(zrow, 0.0)
    dst0 = bass.AP(tensor=xpad.tensor, offset=0, ap=[[N3, 8], [1, 3]])
    nc.sync.dma_start(out=dst0, in_=zrow[:, 0:3])
    for src, h in ((xr, 0), (xi, 1)):
        ld = cvt32.tile([128, W], FP, tag="ld", name=f"ld{h}")
        sap = bass.AP(tensor=src.tensor, offset=0, ap=[[W, 128], [1, W]])
        nc.sync.dma_start(out=ld, in_=sap)
        cv = cvt16.tile([128, W], BF, tag="cv", name=f"cv{h}")
        nc.vector.tensor_copy(cv, ld)
        dap = bass.AP(tensor=xpad.tensor, offset=h * N3 + 3,
                      ap=[[2 * N3, 4], [W, 32], [1, W]])
        nc.sync.dma_start(out=dap, in_=cv)

    # -- deferred output stage: accumulate mean-track + w_out in PSUM --
    def emit_out(feats, u_keep, anchor, rp, s):
        t0s = s * SUP * CH
        for g in range(NG):
            ot = opool.tile([4, 2 * CH], FP, tag="ot", name=f"ot{g}")
            for j in range(2):
                k = 2 * g + j
                sl = slice(j * CH, (j + 1) * CH)
                op_ps = sp_pool.tile([4, CH], FP, tag="va",
                                     name=f"op{g}j{j}",
                                     padded_shape=[128, CH])
                nc.tensor.matmul(out=op_ps, lhsT=mwin,
                                 rhs=feats[:, k * CH:(k + 1) * CH],
                                 start=True, stop=False)
                for li, l in enumerate((1, 3, 5)):
                    nc.tensor.matmul(
                        out=op_ps,
                        lhsT=mw[:, 4 * li: 4 * li + 4],
                        rhs=u_keep[g][li][:, sl],
                        start=False, stop=False)
                nc.tensor.matmul(out=op_ps, lhsT=wout,
                                 rhs=anchor[g][:, sl],
                                 start=False, stop=True)
                nc.vector.tensor_copy(ot[:, sl], op_ps)
            # planar store: rows (A-re, A-im, B-re, B-im)
            t0 = t0s + 2 * g * CH
            dst = bass.AP(tensor=out.tensor,
                          offset=rp * 2 * TPR + t0,
                          ap=[[4 * TPR, 2], [TPR, 2], [1, 2 * CH]])
            nc.sync.dma_start(out=dst, in_=ot)

    # ---------------- main loops ----------------
    pending = None                       # previous super's output tail
    for rp in range(2):                  # row-pair: A=row rp, B=row 2+rp
        for s in range(spr):
            t0s = s * SUP * CH
            # -- feats for the whole super-chunk: [16, SUP*CH] bf16 --
            feats = fpool.tile([16, SUP * CH], BF, tag="feats")
            for b in (0, 1):             # half: A rows / B rows
                src = bass.AP(tensor=xpad.tensor,
                              offset=(rp + 2 * b) * 2 * N3 + t0s,
                              ap=[[N3, 2], [1, 4], [1, SUP * CH]])
                nc.sync.dma_start(out=feats[b * 8:(b + 1) * 8, :], in_=src)

            def fch(k):
                return feats[:, k * CH:(k + 1) * CH]

            # -- stage 0: w_in matmuls -> a0 anchors --
            zps = []                     # per group: [j0_tile, j1_tile]
            for g in range(NG):
                zpj = []
                for j in range(2):
                    zp = zp_pool.tile([128, CH], FP, tag="zp",
                                      name=f"zp0g{g}j{j}")
                    nc.tensor.matmul(out=zp, lhsT=win, rhs=fch(2 * g + j),
                                     start=True, stop=True)
                    zpj.append(zp)
                zps.append(zpj)
            anchor = []
            for g in range(NG):
                a0 = anchors.tile([128, 2 * CH], BF, tag="anc",
                                  name=f"a0g{g}")
                for j in range(2):
                    asl = a0[:, j * CH:(j + 1) * CH]
                    nc.vector.tensor_scalar_add(asl, zps[g][j], b_in_col)
                anchor.append(a0)
            cur = list(anchor)           # LN input (SBUF) for even stages
            u_keep = [[None] * 3 for _ in range(NG)]

            # previous super's output stage overlaps this super's start
            if pending is not None:
                emit_out(*pending)

            for i in range(NL):          # LN stages 0..5
                even = (i % 2 == 0)
                # odd stages: materialize zc = zp + bias to SBUF right away
                # (single early PSUM reader -> zp bank recycles fast)
                new_zps = [None] * NG
                # wide tiles spanning a pair of groups (amortize ACT init)
                for pg in range(NG // 2):
                    gpair = (2 * pg, 2 * pg + 1)
                    t = tpool.tile([128, 4 * CH], BF, tag="t",
                                   name=f"t{i}p{pg}")
                    for q, g in enumerate(gpair):
                        tsl = t[:, q * 2 * CH:(q + 1) * 2 * CH]
                        # --- variance path ---
                        vsq = qpool.tile([128, 2 * CH], BF, tag="vsq",
                                         name=f"vsq{i}g{g}")
                        if even:
                            nc.vector.tensor_mul(vsq, cur[g], cur[g])
                        else:
                            for j in range(2):
                                nc.scalar.activation(
                                    out=vsq[:, j * CH:(j + 1) * CH],
                                    in_=zps[g][j], func=AF.Square,
                                    bias=bcol[i - 1], scale=1.0)
                        va = sp_pool.tile([128, 2 * CH], FP, tag="va",
                                          name=f"va{i}g{g}")
                        for j in range(2):
                            nc.tensor.matmul(
                                out=va[:, j * CH:(j + 1) * CH],
                                lhsT=onesg[:, i * 128:(i + 1) * 128],
                                rhs=vsq[:, j * CH:(j + 1) * CH],
                                start=True, stop=True)
                        # rs = gamma / sqrt(var + eps)  (gamma, eps folded)
                        rs = rpool.tile([128, 2 * CH], BF, tag="rs",
                                        name=f"rs{i}g{g}")
                        nc.scalar.activation(out=rs, in_=va,
                                             func=AF.Abs_reciprocal_sqrt,
                                             bias=gscol[i], scale=1.0)
                        # --- normalize ---
                        if even:
                            nc.vector.tensor_mul(tsl, cur[g], rs)
                        else:
                            for j in range(2):
                                nc.vector.scalar_tensor_tensor(
                                    out=tsl[:, j * CH:(j + 1) * CH],
                                    in0=zps[g][j], scalar=bcol[i - 1],
                                    in1=rs[:, j * CH:(j + 1) * CH],
                                    op0=OP.add, op1=OP.mult)
                    # --- prelu over the whole pair (one wide ACT op) ---
                    u = upool.tile([128, 4 * CH], BF, tag="u",
                                   name=f"u{i}p{pg}")
                    nc.scalar.activation(out=u, in_=t, func=AF.Prelu,
                                         bias=ecol[i], scale=1.0,
                                         alpha=acol[i])
                    for q, g in enumerate(gpair):
                        usl = u[:, q * 2 * CH:(q + 1) * 2 * CH]
                        if not even:
                            u_keep[g][i // 2] = usl
                        # --- dense matmul ---
                        zpj = []
                        for j in range(2):
                            zp = zp_pool.tile([128, CH], FP, tag="zp",
                                              name=f"zp{i + 1}g{g}j{j}")
                            nc.tensor.matmul(
                                out=zp,
                                lhsT=wd[:, i * 128:(i + 1) * 128],
                                rhs=usl[:, j * CH:(j + 1) * CH],
                                start=True, stop=True)
                            zpj.append(zp)
                        new_zps[g] = zpj
                zps = new_zps
                if not even:             # block boundary: materialize Z
                    nxt = []
                    for g in range(NG):
                        zb = anchors.tile([128, 2 * CH], BF, tag="anc",
                                          name=f"zb{i}g{g}")
                        for j in range(2):
                            nc.vector.scalar_tensor_tensor(
                                out=zb[:, j * CH:(j + 1) * CH],
                                in0=zps[g][j], scalar=bcol[i],
                                in1=anchor[g][:, j * CH:(j + 1) * CH],
                                op0=OP.add, op1=OP.add)
                        nxt.append(zb)
                    anchor = nxt
                    cur = list(anchor)

            pending = (feats, u_keep, anchor, rp, s)
    emit_out(*pending)
    ctx.close()


def _host_pack(inputs):
    """Build the shared (replicated) packed-weight arrays (bf16-ready)."""
    w_in = np.asarray(inputs["w_in"], np.float64)        # [8, 64]
    dense_w = np.asarray(inputs["dense_w"], np.float64)  # [6, 64, 64]
    w_out = np.asarray(inputs["w_out"], np.float64)      # [64, 2]
    ln_gamma = np.asarray(inputs["ln_gamma"], np.float32)
    ln_beta = np.asarray(inputs["ln_beta"], np.float32)
    alpha = np.asarray(inputs["alpha"], np.float32)
    b_in = np.asarray(inputs["b_in"], np.float64)
    dense_b = np.asarray(inputs["dense_b"], np.float64)

    P = np.eye(F) - np.ones((F, F)) / F                  # centering
    s_out = w_out.sum(axis=0)                            # [2] colsums

    w_in_f = w_in @ P
    b_in_f = b_in @ P
    dense_w_f = np.stack([dense_w[l] @ P for l in range(NL)])
    dense_b_f = np.stack([dense_b[l] @ P for l in range(NL)])

    def bd(a, n=2):
        """block-diag replicate [r, c] -> [n*r, n*c]"""
        r, c = a.shape
        o = np.zeros((n * r, n * c), np.float32)
        for q in range(n):
            o[q * r:(q + 1) * r, q * c:(q + 1) * c] = a
        return o

    wd = np.zeros((128, NL * 128), np.float32)
    for l in range(NL):
        wd[:, l * 128:(l + 1) * 128] = bd(dense_w_f[l])
    win = bd(w_in_f)                                     # [16, 128]
    wout = np.zeros((128, 4), np.float32)
    wout[0:64, 0:2] = w_out
    wout[64:128, 2:4] = w_out
    # LN stats lhsT per layer: column p scaled by 1/(64*gamma_p^2) so the
    # rs plane comes out as gamma/sigma directly
    onesg = np.zeros((128, NL * 128), np.float32)
    for l in range(NL):
        onesg[:, l * 128:(l + 1) * 128] = bd(
            np.tile((1.0 / (F * ln_gamma[l].astype(np.float64) ** 2))[None, :],
                    (F, 1)).astype(np.float32))

    # mean-track lhsT: cols (A-re, A-im, B-re, B-im), scaled by s_out
    def mtrack(wbar):
        r = len(wbar)
        o = np.zeros((2 * r, 4), np.float32)
        o[0:r, 0] = wbar * s_out[0]
        o[0:r, 1] = wbar * s_out[1]
        o[r:2 * r, 2] = wbar * s_out[0]
        o[r:2 * r, 3] = wbar * s_out[1]
        return o

    mwin = mtrack(w_in.mean(axis=1))                     # [16, 4]
    mw = np.zeros((128, 12), np.float32)
    for li, l in enumerate((1, 3, 5)):
        mw[:, 4 * li: 4 * li + 4] = mtrack(dense_w[l].mean(axis=1))

    percol = np.zeros((128, 26), np.float32)
    percol[:, 0] = np.tile(b_in_f, 2)
    for l in range(NL):
        percol[:, 1 + l] = np.tile(dense_b_f[l], 2)
        percol[:, 7 + l] = np.tile(
            (EPS / ln_gamma[l].astype(np.float64) ** 2).astype(np.float32), 2)
        percol[:, 13 + l] = np.tile(ln_beta[l], 2)
        percol[:, 19 + l] = np.tile(alpha[l], 2)

    # host-side constant correction: (mean of each residual-branch bias)*s
    m_const = (b_in.mean() + dense_b[1].mean() + dense_b[3].mean()
               + dense_b[5].mean())
    out_bias = np.asarray(inputs["b_out"], np.float64) + m_const * s_out

    return dict(wd=wd, win=win, wout=wout, mw=mw, mwin=mwin, onesg=onesg,
                percol=percol), out_bias.astype(np.float32)


def build_program(tokens_per_row):
    nc = bacc.Bacc("TRN2")
    ins = {}
    shapes = dict(wd=(128, NL * 128), win=(16, 128), wout=(128, 4),
                  mw=(128, 12), mwin=(16, 4), onesg=(128, NL * 128))
    for name, shp in shapes.items():
        ins[name] = nc.dram_tensor(name, list(shp), BF,
                                   kind="ExternalInput").ap()
    ins["percol"] = nc.dram_tensor("percol", [128, 26], FP,
                                   kind="ExternalInput").ap()
    ins["xr"] = nc.dram_tensor("xr", [4, tokens_per_row], FP,
                               kind="ExternalInput").ap()
    ins["xi"] = nc.dram_tensor("xi", [4, tokens_per_row], FP,
                               kind="ExternalInput").ap()
    outs = {"out": nc.dram_tensor("out", [4, 2, tokens_per_row],
                                  FP, kind="ExternalOutput").ap()}
    with tile.TileContext(nc) as tc:
        build_kernel(tc, outs, ins, tokens_per_row)
    nc.compile()
    return nc


def _to_bf16(a):
    """Round fp32 ndarray to bf16 bit pattern (ml_dtypes if available)."""
    import ml_dtypes
    return a.astype(ml_dtypes.bfloat16)


def _run(inputs, trace=False):
    from concourse.bass_utils import run_bass_kernel_spmd

    x_real = np.ascontiguousarray(np.asarray(inputs["x_real"], np.float32))
    x_imag = np.ascontiguousarray(np.asarray(inputs["x_imag"], np.float32))
    B, N = x_real.shape
    n_cores = 8
    rows_per_core = B // n_cores

    shared, out_bias = _host_pack(inputs)
    shared = {k: (_to_bf16(v) if k != "percol" else v)
              for k, v in shared.items()}
    nc = build_program(N)

    in_maps = []
    for c in range(n_cores):
        m = dict(shared)
        m["xr"] = np.ascontiguousarray(
            x_real[c * rows_per_core:(c + 1) * rows_per_core])
        m["xi"] = np.ascontiguousarray(
            x_imag[c * rows_per_core:(c + 1) * rows_per_core])
        in_maps.append(m)

    res = run_bass_kernel_spmd(nc, in_maps, core_ids=list(range(n_cores)),
                               trace=trace)
    outs_np = [r["out"] for r in res.results]       # each [4, 2, N]
    full = np.concatenate(outs_np, axis=0)          # [B, 2, N]
    re = full[:, 0, :] + out_bias[0] + x_real
    im = full[:, 1, :] + out_bias[1] + x_imag
    return (re + 1j * im).astype(np.complex64), res


def kernel(**inputs):
    return _run(inputs, trace=False)[0]


# revision 33
# speedup vs baseline: 1.2871x; 1.2871x over previous
"""Trainium2 Bass kernel for nn_NeuralNetworkDPD (dense_mlp)  — v2.

Feature-major, 2-token-halves packed on 128 partitions (A rows {0,1},
B rows {2,3}; partitions [0:64)=A-token features, [64:128)=B).

v2 changes vs v1:
  - Mean-centering projection P = I - 11^T/64 folded into every weight
    matrix host-side, so each layer's activations arrive pre-centered:
    the LN mean matmul and the subtract disappear.  The lost means (the
    final  Z @ w_out  needs the uncentered Z) are reconstructed with 4
    tiny matmuls (feats*mean(w_in), u1*mean(W1), u3*mean(W3),
    u5*mean(W5)) accumulated straight into the output PSUM bank with
    colsum(w_out) folded into their weights.
  - bf16 for all SBUF-resident tensors (weights + activations): 2x DVE
    modes, FWL weight loads, faster PE streaming.  PSUM stays fp32.
  - LN variance path: Square(zp + bias) fused on ACT directly from
    PSUM -> va matmul -> Abs_reciprocal_sqrt(va + eps) on ACT (one op,
    one activation table) -> t = (zp + bias) * rs via
    scalar_tensor_tensor on DVE -> Prelu(gamma*t + beta) on ACT.
  - Residual adds fused into the PSUM->SBUF materialize via
    scalar_tensor_tensor: Z = (zp + bias) + Z_prev.
  - Planar DRAM output [4, 2, N] (contiguous stores; real/imag
    interleave done on host) - kills the v1 4-byte-descriptor storm.
  - Inputs converted fp32->bf16 on device (wide reshape + DVE copy),
    feats loaded one dma_start per half per super-chunk.
"""

import sys
from contextlib import ExitStack

sys.path.insert(0, "/opt/trn_rl_repo")

import numpy as np

import concourse.bacc as bacc
import concourse.bass as bass
import concourse.tile as tile
from concourse import mybir

F = 64          # feature width
NL = 6          # chained dense layers
EPS = 1e-3
CH = 512        # tokens per matmul (PSUM bank)
SUP = 8         # chunks per super-chunk (scheduling window)
BF = mybir.dt.bfloat16
FP = mybir.dt.float32
AF = mybir.ActivationFunctionType
OP = mybir.AluOpType

USE_ARS = True  # Abs_reciprocal_sqrt on ACT; False -> Sqrt + DVE recip
USE_GP = False  # GPSIMD offload: measured 8-15us/op + DVE port contention


def build_kernel(tc, outs, ins, tokens_per_row):
    nc = tc.nc
    TPR = tokens_per_row
    cpr = TPR // CH
    spr = cpr // SUP
    assert cpr % SUP == 0
    NG = SUP // 2               # groups (of 2 chunks) per super
    N3 = TPR + 3

    xr, xi = ins["xr"], ins["xi"]       # [4, TPR] fp32
    out = outs["out"]                   # [4, 2, TPR] fp32 planar

    xpad = nc.dram_tensor("xpad", [4, 2, N3], BF, kind="Internal").ap()

    ctx = ExitStack()
    singles = ctx.enter_context(tc.tile_pool(name="singles", bufs=1))
    cvt32 = ctx.enter_context(tc.tile_pool(name="cvt32", bufs=2))
    cvt16 = ctx.enter_context(tc.tile_pool(name="cvt16", bufs=2))
    fpool = ctx.enter_context(tc.tile_pool(name="fpool", bufs=2))
    anchors = ctx.enter_context(tc.tile_pool(name="anchors", bufs=10))
    upool = ctx.enter_context(tc.tile_pool(name="upool", bufs=17))
    qpool = ctx.enter_context(tc.tile_pool(name="qpool", bufs=8))   # vsq/zc
    rpool = ctx.enter_context(tc.tile_pool(name="rpool", bufs=6))   # rs
    tpool = ctx.enter_context(tc.tile_pool(name="tpool", bufs=4))   # t
    opool = ctx.enter_context(tc.tile_pool(name="opool", bufs=4))
    zp_pool = ctx.enter_context(tc.tile_pool(name="zp", bufs=4, space="PSUM"))
    sp_pool = ctx.enter_context(tc.tile_pool(name="sp", bufs=2, space="PSUM"))

    # ---- weights/constants -> SBUF ----
    wd = singles.tile([128, NL * 128], BF)        # folded dense, block-diag
    win = singles.tile([16, 128], BF)             # folded w_in, block-diag
    wout = singles.tile([128, 4], BF)             # w_out block-diag
    mw = singles.tile([128, 12], BF)              # mean-track lhsT l=1,3,5
    mwin = singles.tile([16, 4], BF)              # mean-track lhsT for feats
    onesg = singles.tile([128, NL * 128], BF)     # LN stats lhsT, 1/(64*g^2)
    percol = singles.tile([128, 26], FP)          # per-partition columns
    nc.sync.dma_start(out=wd, in_=ins["wd"])
    nc.sync.dma_start(out=win, in_=ins["win"])
    nc.sync.dma_start(out=wout, in_=ins["wout"])
    nc.sync.dma_start(out=mw, in_=ins["mw"])
    nc.sync.dma_start(out=mwin, in_=ins["mwin"])
    nc.sync.dma_start(out=onesg, in_=ins["onesg"])
    nc.sync.dma_start(out=percol, in_=ins["percol"])

    b_in_col = percol[:, 0:1]
    bcol = [percol[:, 1 + l: 2 + l] for l in range(NL)]       # centered b_l
    gscol = [percol[:, 7 + l: 8 + l] for l in range(NL)]      # eps/gamma^2
    ecol = [percol[:, 13 + l: 14 + l] for l in range(NL)]     # beta
    acol = [percol[:, 19 + l: 20 + l] for l in range(NL)]     # alpha

    # ---- convert x to bf16 into padded DRAM ----
    # [4, TPR] fp32 viewed as [128, TPR//32]: partition p = 32*row + blk
    W = TPR // 32
    zrow = singles.tile([8, 4], BF)
    nc.vector.memset(zrow, 0.0)
    dst0 = bass.AP(tensor=xpad.tensor, offset=0, ap=[[N3, 8], [1, 3]])
    nc.sync.dma_start(out=dst0, in_=zrow[:, 0:3])
    for src, h in ((xr, 0), (xi, 1)):
        ld = cvt32.tile([128, W], FP, tag="ld", name=f"ld{h}")
        sap = bass.AP(tensor=src.tensor, offset=0, ap=[[W, 128], [1, W]])
        nc.sync.dma_start(out=ld, in_=sap)
        cv = cvt16.tile([128, W], BF, tag="cv", name=f"cv{h}")
        nc.vector.tensor_copy(cv, ld)
        dap = bass.AP(tensor=xpad.tensor, offset=h * N3 + 3,
                      ap=[[2 * N3, 4], [W, 32], [1, W]])
        nc.sync.dma_start(out=dap, in_=cv)

    # -- deferred output stage: accumulate mean-track + w_out in PSUM --
    def emit_out(feats, u_keep, anchor, rp, s):
        t0s = s * SUP * CH
        for g in range(NG):
            ot = opool.tile([4, 2 * CH], FP, tag="ot", name=f"ot{g}")
            for j in range(2):
                k = 2 * g + j
                sl = slice(j * CH, (j + 1) * CH)
                op_ps = sp_pool.tile([4, CH], FP, tag="va",
                                     name=f"op{g}j{j}",
                                     padded_shape=[128, CH])
                nc.tensor.matmul(out=op_ps, lhsT=mwin,
                                 rhs=feats[:, k * CH:(k + 1) * CH],
                                 start=True, stop=False)
                for li, l in enumerate((1, 3, 5)):
                    nc.tensor.matmul(
                        out=op_ps,
                        lhsT=mw[:, 4 * li: 4 * li + 4],
                        rhs=u_keep[g][li][:, sl],
                        start=False, stop=False)
                nc.tensor.matmul(out=op_ps, lhsT=wout,
                                 rhs=anchor[g][:, sl],
                                 start=False, stop=True)
                # alternate copy engine so boundary work parallelizes
                if j == 0:
                    nc.scalar.copy(out=ot[:, sl], in_=op_ps)
                else:
                    nc.vector.tensor_copy(ot[:, sl], op_ps)
            # planar store: rows (A-re, A-im, B-re, B-im)
            t0 = t0s + 2 * g * CH
            dst = bass.AP(tensor=out.tensor,
                          offset=rp * 2 * TPR + t0,
                          ap=[[4 * TPR, 2], [TPR, 2], [1, 2 * CH]])
            nc.sync.dma_start(out=dst, in_=ot)

    # ---------------- main loops ----------------
    pending = None                       # previous super's output tail
    for rp in range(2):                  # row-pair: A=row rp, B=row 2+rp
        for s in range(spr):
            t0s = s * SUP * CH
            # -- feats for the whole super-chunk: [16, SUP*CH] bf16 --
            feats = fpool.tile([16, SUP * CH], BF, tag="feats")
            for b in (0, 1):             # half: A rows / B rows
                src = bass.AP(tensor=xpad.tensor,
                              offset=(rp + 2 * b) * 2 * N3 + t0s,
                              ap=[[N3, 2], [1, 4], [1, SUP * CH]])
                nc.sync.dma_start(out=feats[b * 8:(b + 1) * 8, :], in_=src)

            def fch(k):
                return feats[:, k * CH:(k + 1) * CH]

            # -- stage 0: w_in matmuls -> a0 anchors --
            zps = []                     # per group: [j0_tile, j1_tile]
            for g in range(NG):
                zpj = []
                for j in range(2):
                    zp = zp_pool.tile([128, CH], FP, tag="zp",
                                      name=f"zp0g{g}j{j}")
                    nc.tensor.matmul(out=zp, lhsT=win, rhs=fch(2 * g + j),
                                     start=True, stop=True)
                    zpj.append(zp)
                zps.append(zpj)
            anchor = []
            for g in range(NG):
                a0 = anchors.tile([128, 2 * CH], BF, tag="anc",
                                  name=f"a0g{g}")
                for j in range(2):
                    asl = a0[:, j * CH:(j + 1) * CH]
                    if j == 0:
                        nc.scalar.activation(out=asl, in_=zps[g][j],
                                             func=AF.Identity,
                                             bias=b_in_col, scale=1.0)
                    else:
                        nc.vector.tensor_scalar_add(asl, zps[g][j],
                                                    b_in_col)
                anchor.append(a0)
            cur = list(anchor)           # LN input (SBUF) for even stages
            u_keep = [[None] * 3 for _ in range(NG)]

            # previous super's output stage overlaps this super's start
            if pending is not None:
                emit_out(*pending)

            for i in range(NL):          # LN stages 0..5
                even = (i % 2 == 0)
                # odd stages: materialize zc = zp + bias to SBUF right away
                # (single early PSUM reader -> zp bank recycles fast)
                new_zps = [None] * NG
                # wide tiles spanning a pair of groups (amortize ACT init)
                for pg in range(NG // 2):
                    gpair = (2 * pg, 2 * pg + 1)
                    t = tpool.tile([128, 4 * CH], BF, tag="t",
                                   name=f"t{i}p{pg}")
                    for q, g in enumerate(gpair):
                        tsl = t[:, q * 2 * CH:(q + 1) * 2 * CH]
                        # --- variance path ---
                        vsq = qpool.tile([128, 2 * CH], BF, tag="vsq",
                                         name=f"vsq{i}g{g}")
                        if even:
                            nc.vector.tensor_mul(vsq, cur[g], cur[g])
                        else:
                            for j in range(2):
                                nc.scalar.activation(
                                    out=vsq[:, j * CH:(j + 1) * CH],
                                    in_=zps[g][j], func=AF.Square,
                                    bias=bcol[i - 1], scale=1.0)
                        va = sp_pool.tile([128, 2 * CH], FP, tag="va",
                                          name=f"va{i}g{g}")
                        for j in range(2):
                            nc.tensor.matmul(
                                out=va[:, j * CH:(j + 1) * CH],
                                lhsT=onesg[:, i * 128:(i + 1) * 128],
                                rhs=vsq[:, j * CH:(j + 1) * CH],
                                start=True, stop=True)
                        # rs = gamma / sqrt(var + eps)  (gamma, eps folded)
                        rs = rpool.tile([128, 2 * CH], BF, tag="rs",
                                        name=f"rs{i}g{g}")
                        nc.scalar.activation(out=rs, in_=va,
                                             func=AF.Abs_reciprocal_sqrt,
                                             bias=gscol[i], scale=1.0)
                        # --- normalize ---
                        if even:
                            nc.vector.tensor_mul(tsl, cur[g], rs)
                        else:
                            for j in range(2):
                                nc.vector.scalar_tensor_tensor(
                                    out=tsl[:, j * CH:(j + 1) * CH],
                                    in0=zps[g][j], scalar=bcol[i - 1],
                                    in1=rs[:, j * CH:(j + 1) * CH],
                                    op0=OP.add, op1=OP.mult)
                    # --- prelu over the whole pair (one wide ACT op) ---
                    u = upool.tile([128, 4 * CH], BF, tag="u",
                                   name=f"u{i}p{pg}")
                    nc.scalar.activation(out=u, in_=t, func=AF.Prelu,
                                         bias=ecol[i], scale=1.0,
                                         alpha=acol[i])
                    for q, g in enumerate(gpair):
                        usl = u[:, q * 2 * CH:(q + 1) * 2 * CH]
                        if not even:
                            u_keep[g][i // 2] = usl
                        # --- dense matmul ---
                        zpj = []
                        for j in range(2):
                            zp = zp_pool.tile([128, CH], FP, tag="zp",
                                              name=f"zp{i + 1}g{g}j{j}")
                            nc.tensor.matmul(
                                out=zp,
                                lhsT=wd[:, i * 128:(i + 1) * 128],
                                rhs=usl[:, j * CH:(j + 1) * CH],
                                start=True, stop=True)
                            zpj.append(zp)
                        new_zps[g] = zpj
                zps = new_zps
                if not even:             # block boundary: materialize Z
                    nxt = []
                    for g in range(NG):
                        zb = anchors.tile([128, 2 * CH], BF, tag="anc",
                                          name=f"zb{i}g{g}")
                        for j in range(2):
                            nc.vector.scalar_tensor_tensor(
                                out=zb[:, j * CH:(j + 1) * CH],
                                in0=zps[g][j], scalar=bcol[i],
                                in1=anchor[g][:, j * CH:(j + 1) * CH],
                                op0=OP.add, op1=OP.add)
                        nxt.append(zb)
                    anchor = nxt
                    cur = list(anchor)

            pending = (feats, u_keep, anchor, rp, s)
    emit_out(*pending)
    ctx.close()


def _host_pack(inputs):
    """Build the shared (replicated) packed-weight arrays (bf16-ready)."""
    w_in = np.asarray(inputs["w_in"], np.float64)        # [8, 64]
    dense_w = np.asarray(inputs["dense_w"], np.float64)  # [6, 64, 64]
    w_out = np.asarray(inputs["w_out"], np.float64)      # [64, 2]
    ln_gamma = np.asarray(inputs["ln_gamma"], np.float32)
    ln_beta = np.asarray(inputs["ln_beta"], np.float32)
    alpha = np.asarray(inputs["alpha"], np.float32)
    b_in = np.asarray(inputs["b_in"], np.float64)
    dense_b = np.asarray(inputs["dense_b"], np.float64)

    P = np.eye(F) - np.ones((F, F)) / F                  # centering
    s_out = w_out.sum(axis=0)                            # [2] colsums

    w_in_f = w_in @ P
    b_in_f = b_in @ P
    dense_w_f = np.stack([dense_w[l] @ P for l in range(NL)])
    dense_b_f = np.stack([dense_b[l] @ P for l in range(NL)])

    def bd(a, n=2):
        """block-diag replicate [r, c] -> [n*r, n*c]"""
        r, c = a.shape
        o = np.zeros((n * r, n * c), np.float32)
        for q in range(n):
            o[q * r:(q + 1) * r, q * c:(q + 1) * c] = a
        return o

    wd = np.zeros((128, NL * 128), np.float32)
    for l in range(NL):
        wd[:, l * 128:(l + 1) * 128] = bd(dense_w_f[l])
    win = bd(w_in_f)                                     # [16, 128]
    wout = np.zeros((128, 4), np.float32)
    wout[0:64, 0:2] = w_out
    wout[64:128, 2:4] = w_out
    # LN stats lhsT per layer: column p scaled by 1/(64*gamma_p^2) so the
    # rs plane comes out as gamma/sigma directly
    onesg = np.zeros((128, NL * 128), np.float32)
    for l in range(NL):
        onesg[:, l * 128:(l + 1) * 128] = bd(
            np.tile((1.0 / (F * ln_gamma[l].astype(np.float64) ** 2))[None, :],
                    (F, 1)).astype(np.float32))

    # mean-track lhsT: cols (A-re, A-im, B-re, B-im), scaled by s_out
    def mtrack(wbar):
        r = len(wbar)
        o = np.zeros((2 * r, 4), np.float32)
        o[0:r, 0] = wbar * s_out[0]
        o[0:r, 1] = wbar * s_out[1]
        o[r:2 * r, 2] = wbar * s_out[0]
        o[r:2 * r, 3] = wbar * s_out[1]
        return o

    mwin = mtrack(w_in.mean(axis=1))                     # [16, 4]
    mw = np.zeros((128, 12), np.float32)
    for li, l in enumerate((1, 3, 5)):
        mw[:, 4 * li: 4 * li + 4] = mtrack(dense_w[l].mean(axis=1))

    percol = np.zeros((128, 26), np.float32)
    percol[:, 0] = np.tile(b_in_f, 2)
    for l in range(NL):
        percol[:, 1 + l] = np.tile(dense_b_f[l], 2)
        percol[:, 7 + l] = np.tile(
            (EPS / ln_gamma[l].astype(np.float64) ** 2).astype(np.float32), 2)
        percol[:, 13 + l] = np.tile(ln_beta[l], 2)
        percol[:, 19 + l] = np.tile(alpha[l], 2)

    # host-side constant correction: (mean of each residual-branch bias)*s
    m_const = (b_in.mean() + dense_b[1].mean() + dense_b[3].mean()
               + dense_b[5].mean())
    out_bias = np.asarray(inputs["b_out"], np.float64) + m_const * s_out

    return dict(wd=wd, win=win, wout=wout, mw=mw, mwin=mwin, onesg=onesg,
                percol=percol), out_bias.astype(np.float32)


def build_program(tokens_per_row):
    nc = bacc.Bacc("TRN2")
    ins = {}
    shapes = dict(wd=(128, NL * 128), win=(16, 128), wout=(128, 4),
                  mw=(128, 12), mwin=(16, 4), onesg=(128, NL * 128))
    for name, shp in shapes.items():
        ins[name] = nc.dram_tensor(name, list(shp), BF,
                                   kind="ExternalInput").ap()
    ins["percol"] = nc.dram_tensor("percol", [128, 26], FP,
                                   kind="ExternalInput").ap()
    ins["xr"] = nc.dram_tensor("xr", [4, tokens_per_row], FP,
                               kind="ExternalInput").ap()
    ins["xi"] = nc.dram_tensor("xi", [4, tokens_per_row], FP,
                               kind="ExternalInput").ap()
    outs = {"out": nc.dram_tensor("out", [4, 2, tokens_per_row],
                                  FP, kind="ExternalOutput").ap()}
    with tile.TileContext(nc) as tc:
        build_kernel(tc, outs, ins, tokens_per_row)
    nc.compile()
    return nc


def _to_bf16(a):
    """Round fp32 ndarray to bf16 bit pattern (ml_dtypes if available)."""
    import ml_dtypes
    return a.astype(ml_dtypes.bfloat16)


def _run(inputs, trace=False):
    from concourse.bass_utils import run_bass_kernel_spmd

    x_real = np.ascontiguousarray(np.asarray(inputs["x_real"], np.float32))
    x_imag = np.ascontiguousarray(np.asarray(inputs["x_imag"], np.float32))
    B, N = x_real.shape
    n_cores = 8
    rows_per_core = B // n_cores

    shared, out_bias = _host_pack(inputs)
    shared = {k: (_to_bf16(v) if k != "percol" else v)
              for k, v in shared.items()}
    nc = build_program(N)

    in_maps = []
    for c in range(n_cores):
        m = dict(shared)
        m["xr"] = np.ascontiguousarray(
            x_real[c * rows_per_core:(c + 1) * rows_per_core])
        m["xi"] = np.ascontiguousarray(
            x_imag[c * rows_per_core:(c + 1) * rows_per_core])
        in_maps.append(m)

    res = run_bass_kernel_spmd(nc, in_maps, core_ids=list(range(n_cores)),
                               trace=trace)
    outs_np = [r["out"] for r in res.results]       # each [4, 2, N]
    full = np.concatenate(outs_np, axis=0)          # [B, 2, N]
    re = full[:, 0, :] + out_bias[0] + x_real
    im = full[:, 1, :] + out_bias[1] + x_imag
    return (re + 1j * im).astype(np.complex64), res


def kernel(**inputs):
    return _run(inputs, trace=False)[0]
